# revision 15
# baseline (speedup 1.0000x reference)
"""Trainium2 Bass kernel for nn_Attention (B=16, C=8, H=W=512).

Per sample b:
  q = Wq.x + bq   [1,H,W]
  k = Wk.x + bk   [1,H,W]
  v = Wv.x + bv   [C,H,W]
  S[i,j] = sum_w q[i,w] k[j,w]; A = softmax_j(S); out[c,i,w] = sum_j A[i,j] v[c,j,w]

Sharding: data-parallel over batch, 2 samples per core, 8 cores, no collectives.

v3 design notes:
  - bf16 on every PE-facing tensor (fp16 streams at half rate on trn2's PE);
    fp16 only for the final output staging (cheaper output rounding).
  - conv per 8-row-group chunk: 8 full-width v MMs back-to-back, then 16
    narrow qk MMs (M=32, hi/lo weight split) packed into ONE [128,2,512]
    psum tile via col-strip tile_position -- narrow MMs stream concurrently
    on the 4 PE column strips instead of serializing against v MMs.
  - wv columns are (c_out, g)-major so v's psum partitions are (c,g); the
    whole chunk stages into one vg tile and scatters to vplane with 8
    simple per-channel DMAs (contiguous 16-partition source -> full
    128-partition dest, all 16 SBUF ports busy across the 8 DMAs).
  - vplane partition order is p = 8g + rr (chunk-local); the kT
    deinterleave places columns in the matching permuted order, so the
    A-transpose output lines up with vplane with no extra cost (softmax is
    column-permutation invariant).
  - bv folded into the v drain (sum_j A[i,j] == 1), so out drains are a
    pure 1/rowsum scale batched over 2 channels per op.
  - all psum tiles come from one 4-slot x 2-bank pool so s1's conv can
    overlap s0's out phase within the 8 psum banks.
  - x chunk 0's DMA issues before the const DMAs; all x prefetched on sync.
"""

import os
import sys

import numpy as np

B, C, H, W = 16, 8, 512, 512
NCORES = 8
BPC = B // NCORES  # samples per core
P = 128
G = 16  # rows per group (P // C)
NR = H // G  # 32 row-groups per sample
NCH = 4  # x chunks per sample (8 row-groups each)

_CACHE = {}


def _build():
    if "nc" in _CACHE:
        return _CACHE["nc"]
    sys.path.insert(0, "/opt/trn_rl_repo")
    import concourse.bass as bass
    import concourse.tile as tile
    from concourse import bacc, mybir

    f32 = mybir.dt.float32
    bf16 = mybir.dt.bfloat16
    fp16 = mybir.dt.float16
    AF = mybir.ActivationFunctionType
    AX = mybir.AxisListType
    OP = mybir.AluOpType

    nc = bacc.Bacc("TRN2", target_bir_lowering=False, debug=False)

    xg_d = nc.declare_dram_parameter("xg", [BPC, P, NR, W], bf16, isOutput=False)
    wqkh_d = nc.declare_dram_parameter("wqkh", [P, 32], bf16, isOutput=False)
    wqkl_d = nc.declare_dram_parameter("wqkl", [P, 32], bf16, isOutput=False)
    wv_d = nc.declare_dram_parameter("wv", [P, P], bf16, isOutput=False)
    bqk_d = nc.declare_dram_parameter("bqk", [P, 1], f32, isOutput=False)
    bvg_d = nc.declare_dram_parameter("bvg", [P, 1], f32, isOutput=False)
    idb_d = nc.declare_dram_parameter("identb", [P, P], bf16, isOutput=False)
    # out stored in kernel-native layout [b, it, ch, i128, cc, w]; host unpermutes.
    out_d = nc.declare_dram_parameter("out", [BPC, 4, 2, P, 4, W], fp16, isOutput=True)

    with tile.TileContext(nc) as tc:
        with (
            tc.tile_pool(name="consts", bufs=1) as consts,
            tc.tile_pool(name="xq", bufs=7) as xq_pool,
            tc.tile_pool(name="vg", bufs=3) as vg_pool,
            tc.tile_pool(name="vplane", bufs=8) as vp_pool,
            tc.tile_pool(name="qksb", bufs=8) as qk_pool,
            tc.tile_pool(name="qkt", bufs=16) as qkt_pool,
            tc.tile_pool(name="esb", bufs=8) as e_pool,
            tc.tile_pool(name="atsb", bufs=8) as at_pool,
            tc.tile_pool(name="osb", bufs=3) as o_pool,
            tc.tile_pool(name="stats", bufs=24) as st_pool,
            tc.tile_pool(name="ps", bufs=4, space="PSUM") as ps_pool,
        ):
            # ---- input DMAs: first x chunk first, then consts, then the rest
            xt = {}
            xt[(0, 0)] = xq_pool.tile([P, 8, W], bf16, tag="xq", name="xq")
            nc.sync.dma_start(xt[(0, 0)][:], xg_d.ap()[0][:, 0:8, :])

            wv = consts.tile([P, P], bf16)
            nc.scalar.dma_start(wv[:], wv_d.ap())
            wqkh = consts.tile([P, 32], bf16)
            nc.scalar.dma_start(wqkh[:], wqkh_d.ap())
            wqkl = consts.tile([P, 32], bf16)
            nc.scalar.dma_start(wqkl[:], wqkl_d.ap())
            bqk = consts.tile([P, 1], f32)
            nc.scalar.dma_start(bqk[:], bqk_d.ap())
            bvg = consts.tile([P, 1], f32)
            nc.scalar.dma_start(bvg[:], bvg_d.ap())
            idb = consts.tile([P, P], bf16)
            nc.scalar.dma_start(idb[:], idb_d.ap())

            for b, ci in [(0, 1), (0, 2), (0, 3), (1, 0), (1, 1), (1, 2), (1, 3)]:
                xt[(b, ci)] = xq_pool.tile([P, 8, W], bf16, tag="xq", name="xq")
                nc.sync.dma_start(
                    xt[(b, ci)][:], xg_d.ap()[b][:, 8 * ci : 8 * ci + 8, :]
                )

            cyc = {"v": 0, "o": 0}

            def new_state():
                return {"vplane": None, "qk_sb": [], "qt": [], "kt": [],
                        "e_sb": [], "rs": []}

            def conv_chunk(b, ci, st):
                if st["vplane"] is None:
                    st["vplane"] = [
                        vp_pool.tile([P, C, W], bf16, tag="vplane", name=f"vp{i}")
                        for i in range(4)
                    ]
                xq = xt[(b, ci)]
                # v conv: 8 full-width MMs; psum partitions are (c,g) because
                # wv's columns are (c,g)-major. Whole chunk stages into vg.
                vgt = vg_pool.tile([P, 8, W], bf16, tag="vg", name="vg")
                for k in range(4):
                    psv = ps_pool.tile([P, 2, W], f32, tag="ps", name="psv")
                    for rr in range(2):
                        nc.tensor.matmul(
                            psv[:, rr, :], wv[:], xq[:, 2 * k + rr, :],
                            start=True, stop=True,
                        )
                    # drain with bv folded in (bias per partition = bv[c])
                    if cyc["v"] % 2 == 0:
                        nc.scalar.activation(
                            vgt[:, 2 * k : 2 * k + 2, :], psv[:],
                            AF.Identity, bias=bvg[:],
                        )
                    else:
                        nc.vector.tensor_scalar(
                            vgt[:, 2 * k : 2 * k + 2, :], psv[:],
                            bvg[:], None, op0=OP.add,
                        )
                    cyc["v"] += 1
                # scatter: vplane[ci] partition p = 8g+rr <- vg[16c+g, rr, :]
                for c in range(C):
                    eng = (nc.sync, nc.gpsimd)[c % 2]
                    eng.dma_start(
                        st["vplane"][ci][:, c, :],
                        vgt[16 * c : 16 * c + 16, :, :],
                    )
                # qk conv: 16 narrow MMs into one [128,2,512] psum tile
                psqk = ps_pool.tile([P, 2, W], f32, tag="ps", name="psqk")
                for k in range(8):
                    m, rr = k % 4, k // 4
                    for wpart, stt, sp in ((wqkh, True, False), (wqkl, False, True)):
                        nc.tensor.matmul(
                            psqk[32 * m : 32 * m + 32, rr, :],
                            wpart[:],
                            xq[:, k, :],
                            start=stt,
                            stop=sp,
                            tile_position=(0, 32 * m),
                            skip_group_check=True,
                        )
                sb = qk_pool.tile([P, 2, W], bf16, tag="qksb", name="qksb")
                nc.scalar.activation(sb[:], psqk[:], AF.Identity, bias=bqk[:])
                st["qk_sb"].append(sb)

            def qkT(b, st):
                # transpose q/k -> qT[wt], kT[wt]: [128=w, 512=i] bf16.
                # qT columns natural: i = 64t+16m+g (t = 2ci+rr2).
                # kT columns permuted to match vplane: col = 128ci+8g+4rr2+m.
                for wt in range(4):
                    ptr = ps_pool.tile([P, 8, P], bf16, tag="ps", name="ptr")
                    for t in range(8):
                        ci, rr2 = t // 2, t % 2
                        nc.tensor.transpose(
                            ptr[:, t, :],
                            st["qk_sb"][ci][:, rr2, P * wt : P * wt + P],
                            idb[:],
                        )
                    csrc = ptr[:].rearrange(
                        "p (ci rr2) (m qk g) -> p qk ci rr2 m g",
                        ci=4, rr2=2, m=4, qk=2, g=G,
                    )
                    qt = qkt_pool.tile([P, W], bf16, tag="qkt", name="qt")
                    kt = qkt_pool.tile([P, W], bf16, tag="qkt", name="kt")
                    nc.vector.tensor_copy(
                        qt[:].rearrange(
                            "p (ci rr2 m g) -> p ci rr2 m g", ci=4, rr2=2, m=4
                        ),
                        csrc[:, 0],
                    )
                    nc.scalar.copy(
                        kt[:].rearrange(
                            "p (ci g rr2 m) -> p ci rr2 m g", ci=4, g=G, rr2=2
                        ),
                        csrc[:, 1],
                    )
                    st["qt"].append(qt)
                    st["kt"].append(kt)

            def s_exp(b, st, it):
                pss = ps_pool.tile([P, W], f32, tag="ps", name="pss")
                for wt in range(4):
                    nc.tensor.matmul(
                        pss[:],
                        st["qt"][wt][:, P * it : P * it + P],
                        st["kt"][wt][:],
                        start=(wt == 0),
                        stop=(wt == 3),
                    )
                esb = e_pool.tile([P, W], bf16, tag="esb", name="esb")
                sm = st_pool.tile([P, 1], f32, tag="st", name="sm")
                mx = st_pool.tile([P, 1], f32, tag="st", name="mx")
                nc.vector.reduce_max(mx[:], pss[:], axis=AX.X, negate=True)
                nc.scalar.activation(
                    esb[:], pss[:], AF.Exp, bias=mx[:], accum_out=sm[:]
                )
                rs = st_pool.tile([P, 1], f32, tag="st", name="rs")
                nc.vector.reciprocal(rs[:], sm[:])
                st["e_sb"].append(esb)
                st["rs"].append(rs)

            def a_T(b, st):
                for jt in range(4):
                    psa = ps_pool.tile([P, W], bf16, tag="ps", name="psa")
                    for it in range(4):
                        nc.tensor.transpose(
                            psa[:, P * it : P * it + P],
                            st["e_sb"][it][:, P * jt : P * jt + P],
                            idb[:],
                        )
                    atsb = at_pool.tile([P, W], bf16, tag="atsb", name="atsb")
                    nc.vector.tensor_copy(atsb[:], psa[:])
                    st.setdefault("at", []).append(atsb)

            def out_tile(b, st, it, ch):
                osb = o_pool.tile([P, 4, W], fp16, tag="osb", name="osb")
                psos = [
                    ps_pool.tile([P, 2, W], f32, tag="ps", name="pso")
                    for _ in range(2)
                ]
                for jt in range(4):
                    lhsT = st["at"][jt][:, P * it : P * it + P]
                    for q in range(4):
                        c = 4 * ch + q
                        nc.tensor.matmul(
                            psos[q // 2][:, q % 2, :],
                            lhsT,
                            st["vplane"][jt][:, c, :],
                            start=(jt == 0),
                            stop=(jt == 3),
                            skip_group_check=True,
                        )
                for half in range(2):
                    dst = osb[:, 2 * half : 2 * half + 2, :]
                    if cyc["o"] % 2 == 0:
                        nc.scalar.activation(
                            dst, psos[half][:], AF.Copy, scale=st["rs"][it][:]
                        )
                    else:
                        nc.vector.tensor_scalar(
                            dst, psos[half][:], st["rs"][it][:], None, op0=OP.mult
                        )
                    cyc["o"] += 1
                nc.sync.dma_start(out_d.ap()[b, it, ch], osb[:])

            # ---- emission schedule
            s0 = new_state()
            for ci in range(NCH):
                conv_chunk(0, ci, s0)
            qkT(0, s0)
            for it in range(4):
                s_exp(0, s0, it)
            a_T(0, s0)
            s1 = new_state()
            out_tile(0, s0, 0, 0)
            out_tile(0, s0, 0, 1)
            conv_chunk(1, 0, s1)
            out_tile(0, s0, 1, 0)
            out_tile(0, s0, 1, 1)
            conv_chunk(1, 1, s1)
            out_tile(0, s0, 2, 0)
            out_tile(0, s0, 2, 1)
            conv_chunk(1, 2, s1)
            out_tile(0, s0, 3, 0)
            conv_chunk(1, 3, s1)
            out_tile(0, s0, 3, 1)
            qkT(1, s1)
            for it in range(4):
                s_exp(1, s1, it)
            a_T(1, s1)
            for it in range(4):
                out_tile(1, s1, it, 0)
                out_tile(1, s1, it, 1)

    nc.compile()
    _CACHE["nc"] = nc
    return nc


def _make_consts(Wq, bq, Wk, bk, Wv, bv):
    wqk = np.zeros((P, 32), np.float32)
    for g in range(G):
        for c in range(C):
            wqk[g * C + c, g] = Wq[0, c]
            wqk[g * C + c, 16 + g] = Wk[0, c]
    # v weights: input rows (g,c_in)-major, output cols (c_out,g)-major
    wv = np.zeros((P, P), np.float32)
    for g in range(G):
        for ci in range(C):
            for co in range(C):
                wv[g * C + ci, co * G + g] = Wv[co, ci]
    bqk = np.concatenate([np.full(16, bq[0]), np.full(16, bk[0])] * 4).astype(
        np.float32
    )[:, None]
    bvg = np.repeat(bv.astype(np.float32), G)[:, None]  # partition (c,g)
    import ml_dtypes

    eyeb = np.eye(P).astype(ml_dtypes.bfloat16)
    wqkh = wqk.astype(ml_dtypes.bfloat16)
    wqkl = (wqk - wqkh.astype(np.float32)).astype(ml_dtypes.bfloat16)
    return (wqkh, wqkl, wv.astype(ml_dtypes.bfloat16), bqk, bvg, eyeb)


def _split_x(x):
    import ml_dtypes

    x = np.asarray(x, dtype=np.float32)
    xh = x.astype(ml_dtypes.bfloat16)
    # [B,C,H,W] -> [B, (g c)=128, r=NR, W]   (p = g*C + c, i = r*G + g)
    perm = lambda a: np.ascontiguousarray(
        a.reshape(B, C, NR, G, W).transpose(0, 3, 1, 2, 4).reshape(B, G * C, NR, W)
    )
    return perm(xh)


def make_in_maps(inputs):
    wqkh, wqkl, wv, bqk, bvg, eyeb = _make_consts(
        np.asarray(inputs["Wq"]), np.asarray(inputs["bq"]), np.asarray(inputs["Wk"]),
        np.asarray(inputs["bk"]), np.asarray(inputs["Wv"]), np.asarray(inputs["bv"]),
    )
    xg = _split_x(inputs["x"])
    in_maps = []
    for core in range(NCORES):
        in_maps.append(
            {
                "xg": xg[BPC * core : BPC * core + BPC],
                "wqkh": wqkh,
                "wqkl": wqkl,
                "wv": wv,
                "bqk": bqk,
                "bvg": bvg,
                "identb": eyeb,
            }
        )
    return in_maps


def kernel(x, Wq, bq, Wk, bk, Wv, bv):
    sys.path.insert(0, "/opt/trn_rl_repo")
    from concourse.bass_utils import run_bass_kernel_spmd

    nc = _build()
    in_maps = make_in_maps(
        {"x": x, "Wq": Wq, "bq": bq, "Wk": Wk, "bk": bk, "Wv": Wv, "bv": bv}
    )
    res = run_bass_kernel_spmd(nc, in_maps, core_ids=list(range(NCORES)))
    # unpermute [b, it, ch, i128, cc, w] -> [b, c=4ch+cc, i=128it+i128, w]
    out = np.concatenate(
        [np.asarray(r["out"], dtype=np.float32) for r in res.results], axis=0
    )
    out = out.transpose(0, 2, 4, 1, 3, 5).reshape(B, C, H, W)
    return out


# revision 16
# speedup vs baseline: 1.2399x; 1.2399x over previous
"""Trainium2 Bass kernel for nn_Attention (B=16, C=8, H=W=512).

Per sample b:
  q = Wq.x + bq   [1,H,W]
  k = Wk.x + bk   [1,H,W]
  v = Wv.x + bv   [C,H,W]
  S[i,j] = sum_w q[i,w] k[j,w]; A = softmax_j(S); out[c,i,w] = sum_j A[i,j] v[c,j,w]

Sharding: data-parallel over batch, 2 samples per core, 8 cores, no collectives.

v3 design notes:
  - bf16 on every PE-facing tensor (fp16 streams at half rate on trn2's PE);
    fp16 only for the final output staging (cheaper output rounding).
  - conv per 8-row-group chunk: 8 full-width v MMs back-to-back, then 16
    narrow qk MMs (M=32, hi/lo weight split) packed into ONE [128,2,512]
    psum tile via col-strip tile_position -- narrow MMs stream concurrently
    on the 4 PE column strips instead of serializing against v MMs.
  - wv columns are (c_out, g)-major so v's psum partitions are (c,g); the
    whole chunk stages into one vg tile and scatters to vplane with 8
    simple per-channel DMAs (contiguous 16-partition source -> full
    128-partition dest, all 16 SBUF ports busy across the 8 DMAs).
  - vplane partition order is p = 8g + rr (chunk-local); the kT
    deinterleave places columns in the matching permuted order, so the
    A-transpose output lines up with vplane with no extra cost (softmax is
    column-permutation invariant).
  - bv folded into the v drain (sum_j A[i,j] == 1), so out drains are a
    pure 1/rowsum scale batched over 2 channels per op.
  - all psum tiles come from one 4-slot x 2-bank pool so s1's conv can
    overlap s0's out phase within the 8 psum banks.
  - x chunk 0's DMA issues before the const DMAs; all x prefetched on sync.
"""

import os
import sys

import numpy as np

B, C, H, W = 16, 8, 512, 512
NCORES = 8
BPC = B // NCORES  # samples per core
P = 128
G = 16  # rows per group (P // C)
NR = H // G  # 32 row-groups per sample
NCH = 4  # x chunks per sample (8 row-groups each)

_CACHE = {}


def _build():
    if "nc" in _CACHE:
        return _CACHE["nc"]
    sys.path.insert(0, "/opt/trn_rl_repo")
    import concourse.bass as bass
    import concourse.tile as tile
    from concourse import bacc, mybir

    f32 = mybir.dt.float32
    bf16 = mybir.dt.bfloat16
    fp16 = mybir.dt.float16
    AF = mybir.ActivationFunctionType
    AX = mybir.AxisListType
    OP = mybir.AluOpType

    nc = bacc.Bacc("TRN2", target_bir_lowering=False, debug=False)

    xg_d = nc.declare_dram_parameter("xg", [BPC, P, NR, W], bf16, isOutput=False)
    wqkh_d = nc.declare_dram_parameter("wqkh", [P, 32], bf16, isOutput=False)
    wqkl_d = nc.declare_dram_parameter("wqkl", [P, 32], bf16, isOutput=False)
    wv_d = nc.declare_dram_parameter("wv", [P, P], bf16, isOutput=False)
    bqk_d = nc.declare_dram_parameter("bqk", [P, 1], f32, isOutput=False)
    bvg_d = nc.declare_dram_parameter("bvg", [P, 1], f32, isOutput=False)
    idb_d = nc.declare_dram_parameter("identb", [P, P], bf16, isOutput=False)
    # out stored in kernel-native layout [b, it, ch, i128, cc, w]; host unpermutes.
    out_d = nc.declare_dram_parameter("out", [BPC, 4, 2, P, 4, W], fp16, isOutput=True)

    with tile.TileContext(nc) as tc:
        with (
            tc.tile_pool(name="consts", bufs=1) as consts,
            tc.tile_pool(name="xq", bufs=7) as xq_pool,
            tc.tile_pool(name="vg", bufs=3) as vg_pool,
            tc.tile_pool(name="vplane", bufs=8) as vp_pool,
            tc.tile_pool(name="qksb", bufs=8) as qk_pool,
            tc.tile_pool(name="qkt", bufs=16) as qkt_pool,
            tc.tile_pool(name="esb", bufs=8) as e_pool,
            tc.tile_pool(name="atsb", bufs=8) as at_pool,
            tc.tile_pool(name="osb", bufs=3) as o_pool,
            tc.tile_pool(name="stats", bufs=24) as st_pool,
            tc.tile_pool(name="ps", bufs=4, space="PSUM") as ps_pool,
        ):
            # ---- input DMAs: x chunk 0 first, then consts; later x chunks
            # are issued just-in-time (2-chunk lead) from conv_chunk so the
            # sync queue is never hogged by far-ahead x transfers.
            CHUNKS = [(b, ci) for b in range(BPC) for ci in range(NCH)]
            xt = {}

            def issue_x(b, ci):
                xt[(b, ci)] = xq_pool.tile([P, 8, W], bf16, tag="xq", name="xq")
                nc.sync.dma_start(
                    xt[(b, ci)][:], xg_d.ap()[b][:, 8 * ci : 8 * ci + 8, :]
                )

            issue_x(0, 0)

            wv = consts.tile([P, P], bf16)
            nc.scalar.dma_start(wv[:], wv_d.ap())
            wqkh = consts.tile([P, 32], bf16)
            nc.scalar.dma_start(wqkh[:], wqkh_d.ap())
            wqkl = consts.tile([P, 32], bf16)
            nc.scalar.dma_start(wqkl[:], wqkl_d.ap())
            bqk = consts.tile([P, 1], f32)
            nc.scalar.dma_start(bqk[:], bqk_d.ap())
            bvg = consts.tile([P, 1], f32)
            nc.scalar.dma_start(bvg[:], bvg_d.ap())
            idb = consts.tile([P, P], bf16)
            nc.scalar.dma_start(idb[:], idb_d.ap())
            issue_x(0, 1)

            cyc = {"v": 0, "o": 0, "n": 2}

            def new_state():
                return {"vplane": None, "qk_sb": [], "qt": [], "kt": [],
                        "e_sb": [], "rs": []}

            def conv_chunk(b, ci, st):
                if st["vplane"] is None:
                    st["vplane"] = [
                        vp_pool.tile([P, C, W], bf16, tag="vplane", name=f"vp{i}")
                        for i in range(4)
                    ]
                if cyc["n"] < len(CHUNKS):
                    issue_x(*CHUNKS[cyc["n"]])
                    cyc["n"] += 1
                xq = xt[(b, ci)]
                # v conv: 8 full-width MMs; psum partitions are (c,g) because
                # wv's columns are (c,g)-major. Whole chunk stages into vg.
                vgt = vg_pool.tile([P, 8, W], bf16, tag="vg", name="vg")
                for k in range(4):
                    psv = ps_pool.tile([P, 2, W], f32, tag="ps", name="psv")
                    for rr in range(2):
                        nc.tensor.matmul(
                            psv[:, rr, :], wv[:], xq[:, 2 * k + rr, :],
                            start=True, stop=True,
                        )
                    # drain with bv folded in (bias per partition = bv[c])
                    if cyc["v"] % 2 == 0:
                        nc.scalar.activation(
                            vgt[:, 2 * k : 2 * k + 2, :], psv[:],
                            AF.Identity, bias=bvg[:],
                        )
                    else:
                        nc.vector.tensor_scalar(
                            vgt[:, 2 * k : 2 * k + 2, :], psv[:],
                            bvg[:], None, op0=OP.add,
                        )
                    cyc["v"] += 1
                # scatter: vplane[ci] partition p = 8g+rr <- vg[16c+g, rr, :]
                for c in range(C):
                    eng = (nc.sync, nc.gpsimd)[c % 2]
                    eng.dma_start(
                        st["vplane"][ci][:, c, :],
                        vgt[16 * c : 16 * c + 16, :, :],
                    )
                # qk conv: 16 narrow MMs into one [128,2,512] psum tile
                psqk = ps_pool.tile([P, 2, W], f32, tag="ps", name="psqk")
                for k in range(8):
                    m, rr = k % 4, k // 4
                    for wpart, stt, sp in ((wqkh, True, False), (wqkl, False, True)):
                        nc.tensor.matmul(
                            psqk[32 * m : 32 * m + 32, rr, :],
                            wpart[:],
                            xq[:, k, :],
                            start=stt,
                            stop=sp,
                            tile_position=(0, 32 * m),
                            skip_group_check=True,
                        )
                sb = qk_pool.tile([P, 2, W], bf16, tag="qksb", name="qksb")
                nc.scalar.activation(sb[:], psqk[:], AF.Identity, bias=bqk[:])
                st["qk_sb"].append(sb)

            def qkT(b, st):
                # transpose q/k -> qT[wt], kT[wt]: [128=w, 512=i] bf16.
                # qT columns natural: i = 64t+16m+g (t = 2ci+rr2).
                # kT columns permuted to match vplane: col = 128ci+8g+4rr2+m.
                for wt in range(4):
                    ptr = ps_pool.tile([P, 8, P], bf16, tag="ps", name="ptr")
                    for t in range(8):
                        ci, rr2 = t // 2, t % 2
                        nc.tensor.transpose(
                            ptr[:, t, :],
                            st["qk_sb"][ci][:, rr2, P * wt : P * wt + P],
                            idb[:],
                        )
                    csrc = ptr[:].rearrange(
                        "p (ci rr2) (m qk g) -> p qk ci rr2 m g",
                        ci=4, rr2=2, m=4, qk=2, g=G,
                    )
                    qt = qkt_pool.tile([P, W], bf16, tag="qkt", name="qt")
                    kt = qkt_pool.tile([P, W], bf16, tag="qkt", name="kt")
                    nc.vector.tensor_copy(
                        qt[:].rearrange(
                            "p (ci rr2 m g) -> p ci rr2 m g", ci=4, rr2=2, m=4
                        ),
                        csrc[:, 0],
                    )
                    nc.scalar.copy(
                        kt[:].rearrange(
                            "p (ci g rr2 m) -> p ci rr2 m g", ci=4, g=G, rr2=2
                        ),
                        csrc[:, 1],
                    )
                    st["qt"].append(qt)
                    st["kt"].append(kt)

            def s_exp(b, st, it):
                pss = ps_pool.tile([P, W], f32, tag="ps", name="pss")
                for wt in range(4):
                    nc.tensor.matmul(
                        pss[:],
                        st["qt"][wt][:, P * it : P * it + P],
                        st["kt"][wt][:],
                        start=(wt == 0),
                        stop=(wt == 3),
                    )
                esb = e_pool.tile([P, W], bf16, tag="esb", name="esb")
                sm = st_pool.tile([P, 1], f32, tag="st", name="sm")
                mx = st_pool.tile([P, 1], f32, tag="st", name="mx")
                nc.vector.reduce_max(mx[:], pss[:], axis=AX.X, negate=True)
                nc.scalar.activation(
                    esb[:], pss[:], AF.Exp, bias=mx[:], accum_out=sm[:]
                )
                rs = st_pool.tile([P, 1], f32, tag="st", name="rs")
                nc.vector.reciprocal(rs[:], sm[:])
                st["e_sb"].append(esb)
                st["rs"].append(rs)
                # A-transpose columns for this it, into the shared psum tile
                if it == 0:
                    st["psa"] = ps_pool.tile([P, 4, W], bf16, tag="ps", name="psa")
                for jt in range(4):
                    nc.tensor.transpose(
                        st["psa"][:, jt, P * it : P * it + P],
                        esb[:, P * jt : P * jt + P],
                        idb[:],
                    )

            def a_T(b, st):
                # drain the already-transposed A tiles
                for jt in range(4):
                    atsb = at_pool.tile([P, W], bf16, tag="atsb", name="atsb")
                    nc.vector.tensor_copy(atsb[:], st["psa"][:, jt, :])
                    st.setdefault("at", []).append(atsb)

            def out_tile(b, st, it, ch):
                osb = o_pool.tile([P, 4, W], fp16, tag="osb", name="osb")
                psos = [
                    ps_pool.tile([P, 2, W], f32, tag="ps", name="pso")
                    for _ in range(2)
                ]
                for jt in range(4):
                    lhsT = st["at"][jt][:, P * it : P * it + P]
                    for q in range(4):
                        c = 4 * ch + q
                        nc.tensor.matmul(
                            psos[q // 2][:, q % 2, :],
                            lhsT,
                            st["vplane"][jt][:, c, :],
                            start=(jt == 0),
                            stop=(jt == 3),
                            skip_group_check=True,
                        )
                for half in range(2):
                    dst = osb[:, 2 * half : 2 * half + 2, :]
                    if cyc["o"] % 2 == 0:
                        nc.scalar.activation(
                            dst, psos[half][:], AF.Copy, scale=st["rs"][it][:]
                        )
                    else:
                        nc.vector.tensor_scalar(
                            dst, psos[half][:], st["rs"][it][:], None, op0=OP.mult
                        )
                    cyc["o"] += 1
                nc.sync.dma_start(out_d.ap()[b, it, ch], osb[:])

            # ---- emission schedule: weave s1's conv into s0's attention
            # chain and s0's last out tiles into s1's attention chain so the
            # PE never starves on the softmax dependency chains.
            s0 = new_state()
            s1 = new_state()
            for ci in range(NCH):
                conv_chunk(0, ci, s0)
            qkT(0, s0)
            s_exp(0, s0, 0)
            conv_chunk(1, 0, s1)
            s_exp(0, s0, 1)
            s_exp(0, s0, 2)
            conv_chunk(1, 1, s1)
            s_exp(0, s0, 3)
            a_T(0, s0)
            out_tile(0, s0, 0, 0)
            out_tile(0, s0, 0, 1)
            conv_chunk(1, 2, s1)
            out_tile(0, s0, 1, 0)
            out_tile(0, s0, 1, 1)
            conv_chunk(1, 3, s1)
            out_tile(0, s0, 2, 0)
            out_tile(0, s0, 2, 1)
            qkT(1, s1)
            out_tile(0, s0, 3, 0)
            s_exp(1, s1, 0)
            s_exp(1, s1, 1)
            out_tile(0, s0, 3, 1)
            s_exp(1, s1, 2)
            s_exp(1, s1, 3)
            a_T(1, s1)
            for it in range(4):
                out_tile(1, s1, it, 0)
                out_tile(1, s1, it, 1)

    nc.compile()
    _CACHE["nc"] = nc
    return nc


def _make_consts(Wq, bq, Wk, bk, Wv, bv):
    wqk = np.zeros((P, 32), np.float32)
    for g in range(G):
        for c in range(C):
            wqk[g * C + c, g] = Wq[0, c]
            wqk[g * C + c, 16 + g] = Wk[0, c]
    # v weights: input rows (g,c_in)-major, output cols (c_out,g)-major
    wv = np.zeros((P, P), np.float32)
    for g in range(G):
        for ci in range(C):
            for co in range(C):
                wv[g * C + ci, co * G + g] = Wv[co, ci]
    bqk = np.concatenate([np.full(16, bq[0]), np.full(16, bk[0])] * 4).astype(
        np.float32
    )[:, None]
    bvg = np.repeat(bv.astype(np.float32), G)[:, None]  # partition (c,g)
    import ml_dtypes

    eyeb = np.eye(P).astype(ml_dtypes.bfloat16)
    wqkh = wqk.astype(ml_dtypes.bfloat16)
    wqkl = (wqk - wqkh.astype(np.float32)).astype(ml_dtypes.bfloat16)
    return (wqkh, wqkl, wv.astype(ml_dtypes.bfloat16), bqk, bvg, eyeb)


def _split_x(x):
    import ml_dtypes

    x = np.asarray(x, dtype=np.float32)
    xh = x.astype(ml_dtypes.bfloat16)
    # [B,C,H,W] -> [B, (g c)=128, r=NR, W]   (p = g*C + c, i = r*G + g)
    perm = lambda a: np.ascontiguousarray(
        a.reshape(B, C, NR, G, W).transpose(0, 3, 1, 2, 4).reshape(B, G * C, NR, W)
    )
    return perm(xh)


def make_in_maps(inputs):
    wqkh, wqkl, wv, bqk, bvg, eyeb = _make_consts(
        np.asarray(inputs["Wq"]), np.asarray(inputs["bq"]), np.asarray(inputs["Wk"]),
        np.asarray(inputs["bk"]), np.asarray(inputs["Wv"]), np.asarray(inputs["bv"]),
    )
    xg = _split_x(inputs["x"])
    in_maps = []
    for core in range(NCORES):
        in_maps.append(
            {
                "xg": xg[BPC * core : BPC * core + BPC],
                "wqkh": wqkh,
                "wqkl": wqkl,
                "wv": wv,
                "bqk": bqk,
                "bvg": bvg,
                "identb": eyeb,
            }
        )
    return in_maps


def kernel(x, Wq, bq, Wk, bk, Wv, bv):
    sys.path.insert(0, "/opt/trn_rl_repo")
    from concourse.bass_utils import run_bass_kernel_spmd

    nc = _build()
    in_maps = make_in_maps(
        {"x": x, "Wq": Wq, "bq": bq, "Wk": Wk, "bk": bk, "Wv": Wv, "bv": bv}
    )
    res = run_bass_kernel_spmd(nc, in_maps, core_ids=list(range(NCORES)))
    # unpermute [b, it, ch, i128, cc, w] -> [b, c=4ch+cc, i=128it+i128, w]
    out = np.concatenate(
        [np.asarray(r["out"], dtype=np.float32) for r in res.results], axis=0
    )
    out = out.transpose(0, 2, 4, 1, 3, 5).reshape(B, C, H, W)
    return out
